# revision 2
# baseline (speedup 1.0000x reference)
"""DiffMHA (differential multi-head attention) block on 8 TRN2 NeuronCores.

Problem: B=4, L=1024, D=1024, H=16 heads (DH=64). Three input streams
(e_v, e_a0, e_a1); Q/K projections per stream, scores summed across
streams, causal-masked softmax, context from the v-stream values,
out-projection + residual + LayerNorm.

Sharding: (batch, head-half) -> 8 cores. Core c handles batch c//2 and
heads (c%2)*8 .. (c%2)*8+8. Each core computes its 8 heads' Q/K/V
projections, scores + softmax + context, then an out-projection PARTIAL
over its own 512 channels for ALL 1024 rows (purely local -- no
collective dependency). A pair ReduceScatter(add) sums the two partials
and hands each core exactly its own 512 rows; residual + LayerNorm run
on those rows only. No mid-kernel collectives at all.

Performance structure:
- Channel-major activations; bf16 matmul operands; fp32 PSUM.
- Causal skip: q-half 0 only computes k-tiles 0-3; masking is a
  multiplicative 0/1 bf16 mask applied AFTER exp on diagonal tiles only.
- Softmax 1/sum: Scalar-engine copy of the sum row to SBUF, then a
  fast approximate reciprocal -- keeps the slow multi-pass reciprocal
  off the in-order Vector FIFO that gates the PE.
- Next-fold projection matmuls are emitted BETWEEN attention groups so
  the in-order PE queue never drains on the exp->mask->ctx chain.
- Per-fold context is kept in SBUF (ctx_keep); the out-projection
  partial for each (row-tile, d-half) is ONE PSUM accumulation group
  (4 fold matmuls + ones-row bias rank-1 update), emitted as bubble
  filler inside fold 3's second attention group (rows 0-511) and right
  after it (rows 512-1023).
- The host pre-halves bout so the pair-summed partials carry the bias
  exactly once; the residual is added after the ReduceScatter.
- 4 chunked ReduceScatters (one per own row-tile) pipeline with the
  trailing out-projection work; LayerNorm runs on own rows only.
"""

import os
import sys
import types

import ml_dtypes
import numpy as np

B, L, D, H = 4, 1024, 1024, 16
DH = D // H
HPC = H // 2  # heads per core
C = HPC * DH  # channels per core (512)
SCALE = float(1.0 / np.sqrt(DH))
EPS = 1e-12
NCORES = 8
BF16 = ml_dtypes.bfloat16


def _install_ntff_hook():
    """Recreate antenv.axon_hooks (absent in this image) so
    run_bass_kernel_spmd(trace=True) can capture NTFF profiles."""
    if "antenv.axon_hooks" in sys.modules:
        return
    try:
        from trn_agent_boot.trn_boot import _ntff_profile_via_ctypes

        hook = _ntff_profile_via_ctypes("/opt/axon/libaxon_pjrt.so")
    except Exception:
        hook = None
    mod = types.ModuleType("antenv.axon_hooks")
    mod.get_axon_ntff_profile_hook = lambda: hook
    mod.set_axon_ntff_profile_hook = lambda h: None
    sys.modules["antenv.axon_hooks"] = mod


_install_ntff_hook()

import concourse.bass as bass  # noqa: E402
import concourse.mybir as mybir  # noqa: E402
import concourse.tile as tile  # noqa: E402
from concourse import bacc  # noqa: E402
from concourse.bass_utils import run_bass_kernel_spmd  # noqa: E402

F32 = mybir.dt.float32
BF = mybir.dt.bfloat16
AF = mybir.ActivationFunctionType
ALU = mybir.AluOpType

_NC_CACHE = {}
LAST_RESULT = None

NQF = C // 128  # 4 channel folds per stream (2 heads each)
NLT = L // 128  # 8 l-tiles
NDT = D // 128  # 8 d-tiles (contraction)
NKT = L // 128  # 8 k-tiles
NOT = 4  # own l-tiles (512 rows kept per core)
STREAMS = ("v", "a0", "a1")
PAIRS = [[0, 1], [2, 3], [4, 5], [6, 7]]


def _included_kts(qh):
    """k-tiles that are not fully causally masked for this q-half."""
    return range(4) if qh == 0 else range(NKT)


def _diag_pattern(qh, kt):
    """Index of the 0/1 triangular mask pattern, or None if the tile is
    fully visible (no masking needed)."""
    if qh == 0:
        return kt  # tiles 0-3 straddle the diagonal
    return kt - 4 if kt >= 4 else None


def build_nc():
    nc = bacc.Bacc("TRN2", target_bir_lowering=False, debug=False, num_devices=NCORES)

    # ---- DRAM parameters (per-core shards, host-prepped) ----
    xt = {s: nc.declare_dram_parameter(f"xt_{s}", [D, L], BF, isOutput=False) for s in STREAMS}
    # W fold-sliced on host: [NQF, D, 128]
    wq = {s: nc.declare_dram_parameter(f"wq_{s}", [NQF, D, 128], BF, isOutput=False) for s in STREAMS}
    wk = {s: nc.declare_dram_parameter(f"wk_{s}", [NQF, D, 128], BF, isOutput=False) for s in STREAMS}
    wv = nc.declare_dram_parameter("wv", [D, C], BF, isOutput=False)
    # own-channel rows of Wout, fold-grouped: [128, NQF, D]
    wout = nc.declare_dram_parameter("wout", [128, NQF, D], BF, isOutput=False)
    bq = {s: nc.declare_dram_parameter(f"bq_{s}", [C], F32, isOutput=False) for s in STREAMS}
    bk = {s: nc.declare_dram_parameter(f"bk_{s}", [C], F32, isOutput=False) for s in STREAMS}
    bv = nc.declare_dram_parameter("bv", [1, C], BF, isOutput=False)
    bout_half = nc.declare_dram_parameter("bout_half", [1, D], BF, isOutput=False)
    # 0/1 exp-mask "window" tensor: E[kl, t] = (kl <= t - 384); pattern i
    # is the 512-wide slice starting at 384 - 128*i.
    trimask = nc.declare_dram_parameter("trimask", [128, 896], BF, isOutput=False)
    ev_res = nc.declare_dram_parameter("ev_res", [512, D], BF, isOutput=False)
    gamma = nc.declare_dram_parameter("gamma", [1, D], BF, isOutput=False)
    beta = nc.declare_dram_parameter("beta", [1, D], BF, isOutput=False)
    out = nc.declare_dram_parameter("out", [512, D], F32, isOutput=True)

    with tile.TileContext(nc) as tc:
        with (
            tc.tile_pool(name="persist", bufs=1) as persist,
            tc.tile_pool(name="xtp", bufs=1) as xtp,
            tc.tile_pool(name="wf", bufs=10) as wf,
            tc.tile_pool(name="qkf", bufs=2) as qkf,
            tc.tile_pool(name="small", bufs=4) as small,
            tc.tile_pool(name="attn", bufs=3) as attn_pool,
            tc.tile_pool(name="xstage", bufs=3) as xstage_pool,
            tc.tile_pool(name="ln", bufs=3) as ln_pool,
            tc.tile_pool(name="proj_ps", bufs=2, space="PSUM") as proj_ps,
            tc.tile_pool(name="sc_ps", bufs=4, space="PSUM") as sc_ps,
            tc.tile_pool(name="ctx_ps", bufs=2, space="PSUM") as ctx_ps,
            tc.tile_pool(name="dram", bufs=1, space="DRAM") as dram,
        ):
            # ---- persistent SBUF tensors ----
            vnat = persist.tile([128, NLT, HPC, DH + 1], BF, tag="vnat")
            ctx_keep = persist.tile([128, NQF, L], BF, tag="ctxkeep")
            trimask_sb = persist.tile([128, 896], BF, tag="trimask")
            ones_b = persist.tile([1, 128], BF, tag="ones")
            gb_bc = persist.tile([128, 2, D], BF, tag="gbbc")
            bout_sb = persist.tile([1, D], BF, tag="boutsb")
            bv_sb = persist.tile([1, C], BF, tag="bvsb")
            wout_sb = persist.tile([128, NQF, D], BF, tag="woutsb")
            eps_sb = persist.tile([128, 1], F32, tag="eps")
            wv_sb = persist.tile([128, NDT, C], BF, tag="wvsb")
            ev_sb = persist.tile([128, NOT, D], BF, tag="evsb")
            bq_sb = {
                s: persist.tile([128, NQF], F32, tag=f"bq{s}", name=f"bq_sb_{s}")
                for s in STREAMS
            }
            bk_sb = {
                s: persist.tile([128, NQF], F32, tag=f"bk{s}", name=f"bk_sb_{s}")
                for s in STREAMS
            }

            nc.vector.memset(ones_b[:, :], 1.0)
            nc.vector.memset(eps_sb[:, :], EPS)
            # V ones-column (feeds the softmax-sum rows)
            nc.vector.memset(vnat[:, :, :, DH : DH + 1], 1.0)

            # ---- startup DMAs: wv + xt_v first so the PE starts ASAP ----
            xt_sb = {}
            for s in STREAMS:
                xt_sb[s] = xtp.tile(
                    [128, NDT, L], BF, tag=f"xt{s}", name=f"xt_sb_{s}"
                )
            for dt in range(NDT):
                nc.sync.dma_start(
                    out=wv_sb[:, dt, :], in_=wv[dt * 128 : (dt + 1) * 128, :]
                )
                nc.sync.dma_start(
                    out=xt_sb["v"][:, dt, :], in_=xt["v"][dt * 128 : (dt + 1) * 128, :]
                )
            nc.sync.dma_start(out=bv_sb[:, :], in_=bv[:, :])
            for s in STREAMS:
                nc.sync.dma_start(
                    out=bq_sb[s][:, :], in_=bq[s][:].rearrange("(f p) -> p f", p=128)
                )
                nc.sync.dma_start(
                    out=bk_sb[s][:, :], in_=bk[s][:].rearrange("(f p) -> p f", p=128)
                )
            nc.sync.dma_start(out=trimask_sb[:, :], in_=trimask[:, :])

            # fold-weight prefetch helper (wf holds 2 folds)
            w_tiles = {}

            def emit_wdma(f):
                tiles = {}
                for s in STREAMS:
                    wq_t = wf.tile([128, NDT, 128], BF, tag="w", name=f"wq_{s}{f}")
                    wk_t = wf.tile([128, NDT, 128], BF, tag="w", name=f"wk_{s}{f}")
                    nc.sync.dma_start(
                        out=wq_t[:, :, :],
                        in_=wq[s][f, :, :].rearrange("(dt p) c -> p dt c", p=128),
                    )
                    nc.sync.dma_start(
                        out=wk_t[:, :, :],
                        in_=wk[s][f, :, :].rearrange("(dt p) c -> p dt c", p=128),
                    )
                    tiles[s] = (wq_t, wk_t)
                w_tiles[f] = tiles

            emit_wdma(0)

            # ---- V projection (natural [l, c] layout + ones column) ----
            for lf in range(NLT):
                ps = proj_ps.tile([128, C], F32, tag="proj")
                for dt in range(NDT):
                    nc.tensor.matmul(
                        ps[:, :],
                        xt_sb["v"][:, dt, lf * 128 : (lf + 1) * 128],
                        wv_sb[:, dt, :],
                        start=(dt == 0),
                        stop=False,
                    )
                # + bias via ones-row rank-1 update
                nc.tensor.matmul(
                    ps[:, :],
                    ones_b[:, :],
                    bv_sb[:, :],
                    start=False,
                    stop=True,
                )
                nc.scalar.copy(vnat[:, lf, :, 0:DH], ps[:, :])

            # remaining embeddings + late-needed tensors
            for s in ("a0", "a1"):
                for dt in range(NDT):
                    nc.sync.dma_start(
                        out=xt_sb[s][:, dt, :],
                        in_=xt[s][dt * 128 : (dt + 1) * 128, :],
                    )
            emit_wdma(1)
            nc.sync.dma_start(out=wout_sb[:, :, :], in_=wout[:, :, :])
            nc.sync.dma_start(out=bout_sb[:, :], in_=bout_half[:, :])
            for k in range(NOT):
                nc.sync.dma_start(
                    out=ev_sb[:, k, :], in_=ev_res[k * 128 : (k + 1) * 128, :]
                )
            gsb = small.tile([1, D], BF, tag="gsb", bufs=1)
            bsb = small.tile([1, D], BF, tag="bsb", bufs=1)
            nc.sync.dma_start(out=gsb[:, :], in_=gamma[:, :])
            nc.sync.dma_start(out=bsb[:, :], in_=beta[:, :])
            nc.gpsimd.partition_broadcast(gb_bc[:, 0, :], gsb[:, :])
            nc.gpsimd.partition_broadcast(gb_bc[:, 1, :], bsb[:, :])

            # ReduceScatter DRAM staging: chunk k pairs own-row-tile k
            # (slot 0, rows k*128..) with row-tile 4+k (slot 1).
            rs_in = {}
            rs_out = {}
            for k in range(NOT):
                rs_in[k] = dram.tile(
                    [2, 128, D], BF, name=f"rs_in{k}", tag=f"rsin{k}"
                )
                rs_out[k] = dram.tile(
                    [128, D], BF, name=f"rs_out{k}", tag=f"rsout{k}"
                )

            def emit_proj(f, streams_sel, qtf, ktf):
                for s in streams_sel:
                    wq_t, wk_t = w_tiles[f][s]
                    for which, w_t, b_t, store in (
                        ("q", wq_t, bq_sb[s], qtf),
                        ("k", wk_t, bk_sb[s], ktf),
                    ):
                        dst = qkf.tile(
                            [128, L], BF, tag=f"{which}t{s}", name=f"{which}t_{s}{f}"
                        )
                        for lh in range(2):
                            ps = proj_ps.tile([128, 512], F32, tag="proj")
                            for dt in range(NDT):
                                nc.tensor.matmul(
                                    ps[:, :],
                                    w_t[:, dt, :],
                                    xt_sb[s][:, dt, lh * 512 : (lh + 1) * 512],
                                    start=(dt == 0),
                                    stop=(dt == NDT - 1),
                                )
                            nc.scalar.activation(
                                dst[:, lh * 512 : (lh + 1) * 512],
                                ps[:, :],
                                AF.Identity,
                                bias=b_t[:, f : f + 1],
                            )
                        store[s] = dst

            # out-projection partial for (row-tile lt, d-half dh_i): ONE
            # PSUM group -- 4 own-channel-fold matmuls + half-bias rank-1.
            # Result staged to SBUF bf16, then DMA'd to the RS input slot.
            def emit_outproj(lt, dh_i, x_dst):
                lsl = slice(lt * 128, (lt + 1) * 128)
                dsl = slice(dh_i * 512, (dh_i + 1) * 512)
                ps = proj_ps.tile([128, 512], F32, tag="proj")
                for cf in range(NQF):
                    nc.tensor.matmul(
                        ps[:, :],
                        ctx_keep[:, cf, lsl],
                        wout_sb[:, cf, dsl],
                        start=(cf == 0),
                        stop=False,
                    )
                nc.tensor.matmul(
                    ps[:, :], ones_b[:, :], bout_sb[:, dsl], start=False, stop=True
                )
                nc.scalar.copy(x_dst[:, dsl], ps[:, :])
                slot, k = divmod(lt, NOT)
                nc.sync.dma_start(out=rs_in[k][slot, :, dsl], in_=x_dst[:, dsl])

            def emit_group(f, qh, qtf, ktf, fillers=()):
                """Both heads of the fold for one q-half, with the two
                heads' 64-partition score matmuls emitted back-to-back:
                they target disjoint PE row groups (base partitions 0/64)
                and different PSUM banks, so the array runs them
                concurrently. `fillers` are zero-arg emitters interleaved
                after each k-tile to fill exp->mask->ctx pipeline drains
                on the in-order PE queue."""
                qsl = slice(qh * 512, (qh + 1) * 512)
                kts = list(_included_kts(qh))
                fillers = list(fillers)
                cps = {
                    hh: ctx_ps.tile(
                        [DH + 1, 512], F32, tag="ctx", name=f"cps{hh}"
                    )
                    for hh in range(2)
                }
                for kt_i in kts:
                    ksl = slice(kt_i * 128, (kt_i + 1) * 128)
                    sps = {
                        hh: sc_ps.tile([128, 512], F32, tag="sc", name=f"sps{hh}")
                        for hh in range(2)
                    }
                    for i, s in enumerate(STREAMS):
                        for hh in range(2):
                            p0 = hh * 64
                            nc.tensor.matmul(
                                sps[hh][:, :],
                                ktf[s][p0 : p0 + 64, ksl],
                                qtf[s][p0 : p0 + 64, qsl],
                                start=(i == 0),
                                stop=(i == 2),
                            )
                    pat = _diag_pattern(qh, kt_i)
                    for hh in range(2):
                        attn_sb = attn_pool.tile([128, 512], BF, tag="attn")
                        nc.scalar.activation(
                            attn_sb[:, :], sps[hh][:, :], AF.Exp, scale=SCALE
                        )
                        if pat is not None:
                            off = 384 - 128 * pat
                            nc.vector.tensor_mul(
                                attn_sb[:, :],
                                attn_sb[:, :],
                                trimask_sb[:, off : off + 512],
                            )
                        nc.tensor.matmul(
                            cps[hh][:, :],
                            vnat[:, kt_i, 2 * f + hh, :],
                            attn_sb[:, :],
                            start=(kt_i == kts[0]),
                            stop=(kt_i == kts[-1]),
                        )
                    if fillers:
                        fillers.pop(0)()
                # 1/sum: stage the sum row to SBUF on the Scalar engine
                # (Identity -- no ACT table swap), then a fast approximate
                # reciprocal keeps the slow multi-pass reciprocal off the
                # Vector FIFO that gates the PE.
                for hh in range(2):
                    p0 = hh * 64
                    sum_sb = small.tile([1, 512], F32, tag="sumsb", bufs=1)
                    nc.scalar.copy(sum_sb[:, :], cps[hh][DH : DH + 1, :])
                    inv = small.tile([1, 512], F32, tag="inv", bufs=2)
                    nc.vector.reciprocal_approx_fast(inv[:, :], sum_sb[:, :])
                    inv_bc = small.tile([64, 512], F32, tag="invbc", bufs=2)
                    nc.gpsimd.partition_broadcast(inv_bc[:, :], inv[:, :])
                    nc.vector.tensor_mul(
                        ctx_keep[p0 : p0 + 64, f, qsl], cps[hh][0:DH, :], inv_bc[:, :]
                    )
                for filler in fillers:
                    filler()

            # ---- fold-major schedule with interleaved emission ----
            proj_tiles = {}
            proj_tiles[0] = ({}, {})
            emit_proj(0, STREAMS, *proj_tiles[0])
            x_tiles = {}
            for f in range(NQF):
                qtf, ktf = proj_tiles.pop(f)
                if f + 2 < NQF:
                    emit_wdma(f + 2)
                if f < 3:
                    proj_tiles[f + 1] = ({}, {})

                if f < 3:
                    emit_group(f, 0, qtf, ktf)
                    emit_proj(f + 1, ("v", "a0"), *proj_tiles[f + 1])
                    emit_group(f, 1, qtf, ktf)
                    emit_proj(f + 1, ("a1",), *proj_tiles[f + 1])
                else:
                    # fold 3: out-projection partials are the bubble filler.
                    emit_group(f, 0, qtf, ktf)
                    fillers = []
                    for lt in range(NOT):
                        x_tiles[lt] = xstage_pool.tile(
                            [128, D], BF, tag="xs", name=f"xs{lt}"
                        )
                        for dh_i in range(2):
                            fillers.append(
                                lambda lt=lt, dh_i=dh_i: emit_outproj(
                                    lt, dh_i, x_tiles[lt]
                                )
                            )
                    emit_group(f, 1, qtf, ktf, fillers=fillers)

            # rows 512-1023 partials + chunked ReduceScatter + LN tail
            for k in range(NOT):
                lt = NOT + k
                x_t = xstage_pool.tile([128, D], BF, tag="xs", name=f"xs{lt}")
                for dh_i in range(2):
                    emit_outproj(lt, dh_i, x_t)
                nc.gpsimd.collective_compute(
                    "ReduceScatter",
                    ALU.add,
                    replica_groups=PAIRS,
                    ins=[rs_in[k].opt()],
                    outs=[rs_out[k].opt()],
                )

            for k in range(NOT):
                rs_sb = ln_pool.tile([128, D], BF, tag="rssb")
                nc.sync.dma_start(out=rs_sb[:, :], in_=rs_out[k][:, :])
                x_sb = ln_pool.tile([128, D], F32, tag="x")
                # residual (own rows)
                nc.vector.tensor_add(x_sb[:, :], rs_sb[:, :], ev_sb[:, k, :])
                stats = small.tile([128, 2, 6], F32, tag="stats")
                nc.vector.bn_stats(out=stats[:, 0, :], in_=x_sb[:, 0:512])
                nc.vector.bn_stats(out=stats[:, 1, :], in_=x_sb[:, 512:1024])
                mv = small.tile([128, 2], F32, tag="mv")
                nc.vector.bn_aggr(out=mv[:, :], in_=stats[:, :, :])
                std = small.tile([128, 1], F32, tag="std")
                nc.scalar.activation(std[:, :], mv[:, 1:2], AF.Sqrt, bias=eps_sb[:, :])
                rstd = small.tile([128, 1], F32, tag="rstd")
                nc.vector.reciprocal(rstd[:, :], std[:, :])
                negmb = small.tile([128, 1], F32, tag="negmb")
                nc.vector.scalar_tensor_tensor(
                    negmb[:, :],
                    mv[:, 0:1],
                    -1.0,
                    rstd[:, :],
                    op0=ALU.mult,
                    op1=ALU.mult,
                )
                nc.scalar.activation(
                    x_sb[:, :],
                    x_sb[:, :],
                    AF.Identity,
                    bias=negmb[:, :],
                    scale=rstd[:, :],
                )
                # alternate gamma/beta between Vector and GpSimd so the
                # Vector FIFO (stats + adds) isn't the lone tail engine
                eng = nc.vector if k % 2 == 0 else nc.gpsimd
                eng.tensor_mul(x_sb[:, :], x_sb[:, :], gb_bc[:, 0, :])
                eng.tensor_add(x_sb[:, :], x_sb[:, :], gb_bc[:, 1, :])
                nc.sync.dma_start(
                    out=out[k * 128 : (k + 1) * 128, :], in_=x_sb[:, :]
                )

    nc.compile()
    return nc


def _get_nc():
    if "nc" not in _NC_CACHE:
        _NC_CACHE["nc"] = build_nc()
    return _NC_CACHE["nc"]


def kernel(
    e_v, e_a0, e_a1, Wqv, bqv, Wkv, bkv, Wvv, bvv,
    Wqa0, bqa0, Wka0, bka0, Wqa1, bqa1, Wka1, bka1,
    Wout, bout, ln_gamma, ln_beta, attn_mask,
):
    global LAST_RESULT
    f = np.asarray
    e_v, e_a0, e_a1 = f(e_v), f(e_a0), f(e_a1)
    attn_mask = f(attn_mask)
    c32 = lambda a: np.ascontiguousarray(a, dtype=np.float32)
    cbf = lambda a: np.ascontiguousarray(np.asarray(a, dtype=np.float32).astype(BF16))

    wq_full = {"v": f(Wqv), "a0": f(Wqa0), "a1": f(Wqa1)}
    wk_full = {"v": f(Wkv), "a0": f(Wka0), "a1": f(Wka1)}
    bq_full = {"v": f(bqv), "a0": f(bqa0), "a1": f(bqa1)}
    bk_full = {"v": f(bkv), "a0": f(bka0), "a1": f(bka1)}

    xts = {}
    for b in range(B):
        xts[b] = {
            "v": cbf(e_v[b].T),
            "a0": cbf(e_a0[b].T),
            "a1": cbf(e_a1[b].T),
        }

    # 0/1 exp-mask window from the input mask (scores^T [k, q] layout):
    # E[kl, t] = visibility of k-row kl vs q-col (t - 384); pattern i is
    # the 512-wide slice at offset 384 - 128*i.
    vis = (attn_mask[0, 0] > -0.5).astype(np.float32)  # [q, k] visible=1
    trimask_np = np.zeros((128, 896), dtype=np.float32)
    trimask_np[:, 384:896] = vis[0:512, 0:128].T
    trimask_np = cbf(trimask_np)

    # half bias: both pair partials carry bout/2 so the RS sum is bout
    bout_g = cbf(f(bout) * 0.5).reshape(1, D)

    def fold_slice(w, S):
        # [D, C] slice -> [NQF, D, 128] fold-major
        ws = np.asarray(w[:, S], dtype=np.float32)  # [D, C]
        return np.ascontiguousarray(
            ws.reshape(D, NQF, 128).transpose(1, 0, 2).astype(BF16)
        )

    wout_np = f(Wout)
    in_maps = []
    for c in range(NCORES):
        b, hh = c // 2, c % 2
        S = slice(hh * C, (hh + 1) * C)
        m = {}
        for s in STREAMS:
            m[f"xt_{s}"] = xts[b][s]
            m[f"wq_{s}"] = fold_slice(wq_full[s], S)
            m[f"wk_{s}"] = fold_slice(wk_full[s], S)
            m[f"bq_{s}"] = c32(bq_full[s][S])
            m[f"bk_{s}"] = c32(bk_full[s][S])
        m["wv"] = cbf(f(Wvv)[:, S])
        m["bv"] = cbf(f(bvv)[S]).reshape(1, C)
        # own-channel rows of Wout, fold-grouped -> [128, NQF, D]
        m["wout"] = cbf(
            np.asarray(wout_np[S, :], dtype=np.float32)
            .reshape(NQF, 128, D)
            .transpose(1, 0, 2)
        )
        m["bout_half"] = bout_g
        m["trimask"] = trimask_np
        m["ev_res"] = cbf(e_v[b][hh * 512 : (hh + 1) * 512])
        m["gamma"] = cbf(f(ln_gamma)).reshape(1, D)
        m["beta"] = cbf(f(ln_beta)).reshape(1, D)
        in_maps.append(m)

    nc = _get_nc()
    trace = bool(os.environ.get("KERNEL_TRACE"))
    res = run_bass_kernel_spmd(
        nc, in_maps, core_ids=list(range(NCORES)), trace=trace
    )
    LAST_RESULT = res

    out_full = np.empty((B, L, D), dtype=np.float32)
    for c in range(NCORES):
        b, hh = c // 2, c % 2
        out_full[b, hh * 512 : (hh + 1) * 512, :] = res.results[c]["out"]
    return out_full
